# revision 8
# baseline (speedup 1.0000x reference)
"""Trainium2 Bass kernel for a GPT-style transformer block.

Problem: x[1,4096,768] through LN1 -> QKV -> 12-head attention (no causal
mask) -> out-proj + residual -> LN2 -> 4x MLP (exact gelu) -> proj + residual.

Strategy (8 NeuronCores):
  - Sequence-shard: each core owns 512 tokens for every dense op.
  - Activations kept transposed [feature, token] so weights in natural
    [in,out] layout serve directly as the stationary matmul operand (f32r).
  - Attention: scores computed transposed ([keys, queries]); softmax without
    max-subtraction (scores are small: exp is safe); denominator via an extra
    ones-column on V in the att@V matmul; normalization applied after att@V.
  - K^T and V are AllGather'd across the 8 cores (one fused f32r blob).
  - Flash-style streaming: exp chunks live briefly in SBUF; att@V accumulates
    in PSUM across all 4096 keys.
"""
import numpy as np

import concourse.bass as bass
import concourse.bacc as bacc
import concourse.tile as tile
from concourse import mybir
from concourse.bass_utils import run_bass_kernel_spmd
from concourse.masks import make_identity

f32 = mybir.dt.float32
f32r = mybir.dt.float32r
AF = mybir.ActivationFunctionType
ALU = mybir.AluOpType

N_CORES = 8
T, C, H, D = 4096, 768, 12, 64
TL = T // N_CORES            # 512 tokens per core
NTB = TL // 128              # 4 token tiles
NKT = C // 128               # 6 feature tiles
HID = 4 * C                  # 3072
NHT = HID // 128             # 24
NPAIR = H // 2               # 6 head pairs
EPS = 1e-5

KT_WORDS = C * TL            # K^T region words in AG blob
V_WORDS = TL * C             # V region words
BLOB = KT_WORDS + V_WORDS


def _layernorm_nat(nc, pool, x_ap, g_bc, b_bc, eps_t, out_ap, tag):
    """LayerNorm in natural [token-part, feature-free] layout.
    x_ap/out_ap: [128, NTB, C]."""
    for tb in range(NTB):
        xin = x_ap[:, tb, :]
        stats = pool.tile([128, 3, 6], f32, name=f"{tag}_st{tb}", tag=f"{tag}_st")
        for s in range(3):
            nc.vector.bn_stats(out=stats[:, s, :], in_=xin[:, s * 256:(s + 1) * 256])
        mv = pool.tile([128, 2], f32, name=f"{tag}_mv{tb}", tag=f"{tag}_mv")
        nc.vector.bn_aggr(out=mv[:], in_=stats[:])
        rstd = pool.tile([128, 1], f32, name=f"{tag}_rs{tb}", tag=f"{tag}_rs")
        nc.scalar.activation(out=rstd[:], in_=mv[:, 1:2], func=AF.Sqrt,
                             bias=eps_t[:], scale=1.0)
        nc.vector.reciprocal(out=rstd[:], in_=rstd[:])
        tmp = pool.tile([128, C], f32, name=f"{tag}_tmp{tb}", tag=f"{tag}_tmp")
        nc.vector.tensor_scalar(out=tmp[:], in0=xin, scalar1=mv[:, 0:1],
                                scalar2=rstd[:], op0=ALU.subtract, op1=ALU.mult)
        nc.vector.tensor_mul(out=tmp[:], in0=tmp[:], in1=g_bc[:])
        nc.vector.tensor_add(out=out_ap[:, tb, :], in0=tmp[:], in1=b_bc[:])


def _transpose_to_T(nc, psum_pool, nat_ap, dst_ap, ident, tag):
    """[128, NTB, C] natural -> [128, NKT, TL] transposed (dst f32r)."""
    for k in range(NKT):
        for tb in range(NTB):
            pt = psum_pool.tile([128, 128], f32, name=f"{tag}_{k}_{tb}", tag="tp")
            nc.tensor.transpose(pt[:], nat_ap[:, tb, k * 128:(k + 1) * 128], ident[:])
            nc.vector.tensor_copy(out=dst_ap[:, k, tb * 128:(tb + 1) * 128], in_=pt[:])


def build(reps=1):
    nc = bacc.Bacc("TRN2", target_bir_lowering=False, debug=False,
                   num_devices=N_CORES)

    x_in = nc.dram_tensor("x_sh", [TL, C], f32, kind="ExternalInput")
    w_attn = nc.dram_tensor("w_attn", [C, 3 * C], f32, kind="ExternalInput")
    b_attn = nc.dram_tensor("b_attn", [3 * C], f32, kind="ExternalInput")
    w_ao = nc.dram_tensor("w_ao", [C, C], f32, kind="ExternalInput")
    b_ao = nc.dram_tensor("b_ao", [C], f32, kind="ExternalInput")
    ln1_g = nc.dram_tensor("ln1_g", [C], f32, kind="ExternalInput")
    ln1_b = nc.dram_tensor("ln1_b", [C], f32, kind="ExternalInput")
    ln2_g = nc.dram_tensor("ln2_g", [C], f32, kind="ExternalInput")
    ln2_b = nc.dram_tensor("ln2_b", [C], f32, kind="ExternalInput")
    w_fc = nc.dram_tensor("w_fc", [C, HID], f32, kind="ExternalInput")
    b_fc = nc.dram_tensor("b_fc", [HID], f32, kind="ExternalInput")
    w_proj = nc.dram_tensor("w_proj", [HID, C], f32, kind="ExternalInput")
    b_proj = nc.dram_tensor("b_proj", [C], f32, kind="ExternalInput")
    out = nc.dram_tensor("out_sh", [TL, C], f32, kind="ExternalOutput")

    def bcast_row(pool, src, name, n=C):
        """[n] dram -> [128, n] sbuf broadcast along partitions."""
        t = pool.tile([128, n], f32, name=name)
        src_b = bass.AP(tensor=src.ap().tensor, offset=0, ap=[[0, 128], [1, n]])
        nc.gpsimd.dma_start(out=t[:], in_=src_b)
        return t

    def bias_cols(pool, src, nblk, name):
        """[nblk*128] dram -> [128, nblk] sbuf; col j = bias of feature blk j."""
        t = pool.tile([128, nblk], f32, name=name)
        src_b = bass.AP(tensor=src.ap().tensor, offset=0, ap=[[1, 128], [128, nblk]])
        nc.sync.dma_start(out=t[:], in_=src_b)
        return t

    with tile.TileContext(nc) as tc:
      with tc.tile_pool(name="const", bufs=1) as cons, \
           tc.tile_pool(name="ps_glob", bufs=1, space="PSUM") as psg:
        ident = cons.tile([128, 128], f32)
        make_identity(nc, ident)
        ones1_f = cons.tile([1, 128], f32)
        nc.vector.memset(ones1_f[:], 1.0)
        ones1 = cons.tile([1, 128], f32r)
        nc.vector.tensor_copy(out=ones1[:], in_=ones1_f[:])
        eps_t = cons.tile([128, 1], f32)
        nc.vector.memset(eps_t[:], EPS)
        g1_bc = bcast_row(cons, ln1_g, "g1_bc")
        b1_bc = bcast_row(cons, ln1_b, "b1_bc")
        g2_bc = bcast_row(cons, ln2_g, "g2_bc")
        b2_bc = bcast_row(cons, ln2_b, "b2_bc")
        bv_bc = cons.tile([128, C], f32, name="bv_bc")
        bv_src = bass.AP(tensor=b_attn.ap().tensor, offset=2 * C,
                         ap=[[0, 128], [1, C]])
        nc.gpsimd.dma_start(out=bv_bc[:], in_=bv_src)
        bqk_sb = bias_cols(cons, b_attn, 12, "bqk_sb")   # q: 0-5, k: 6-11
        bao_sb = bias_cols(cons, b_ao, NKT, "bao_sb")
        bfc_sb = bias_cols(cons, b_fc, NHT, "bfc_sb")
        bpr_sb = bias_cols(cons, b_proj, NKT, "bpr_sb")

        for rep in range(reps):
          with tc.tile_pool(name=f"pers{rep}", bufs=1) as pers, \
               tc.tile_pool(name=f"dram{rep}", bufs=1, space="DRAM") as dram:
            x_t = pers.tile([128, NTB, C], f32, name="x_t")
            nc.sync.dma_start(
                out=x_t[:],
                in_=x_in.ap().rearrange("(tb p) f -> p tb f", p=128))
            x2_t = pers.tile([128, NTB, C], f32, name="x2_t")
            qT = pers.tile([128, NKT, TL], f32r, name="qT")
            yT = pers.tile([128, NKT, TL], f32r, name="yT")

            ag_in = dram.tile([BLOB], f32r, name="ag_in")
            ag_out = dram.tile([N_CORES, BLOB], f32r, name="ag_out",
                               addr_space="Shared")

            # ---------------- phase 1-5: LN1, qkT, V, AG send ----------
            with tc.tile_pool(name=f"ph1{rep}", bufs=1) as p1, \
                 tc.tile_pool(name=f"ph1s{rep}", bufs=2) as p1s, \
                 tc.tile_pool(name=f"ps1{rep}", bufs=2, space="PSUM") as ps1:
                ln1 = p1.tile([128, NTB, C], f32, name="ln1")
                _layernorm_nat(nc, p1s, x_t, g1_bc, b1_bc, eps_t, ln1, "ln1")
                ln1T = p1.tile([128, NKT, TL], f32r, name="ln1T")
                _transpose_to_T(nc, ps1, ln1, ln1T, ident, "t1")

                wqk = []
                for k in range(NKT):
                    w = p1.tile([128, 2 * C], f32r, name=f"wqk{k}")
                    nc.sync.dma_start(
                        out=w[:],
                        in_=w_attn[k * 128:(k + 1) * 128, 0:2 * C].bitcast(f32r))
                    wqk.append(w)
                kT_send = p1.tile([128, NKT, TL], f32r, name="kT_send")
                for m in range(12):
                    ps_m = ps1.tile([128, TL], f32, name=f"qk{m}", tag="qkps")
                    for k in range(NKT):
                        nc.tensor.matmul(ps_m[:], wqk[k][:, m * 128:(m + 1) * 128],
                                         ln1T[:, k, :],
                                         start=(k == 0), stop=(k == NKT - 1))
                    dst = qT[:, m, :] if m < NKT else kT_send[:, m - NKT, :]
                    nc.vector.tensor_scalar_add(out=dst, in0=ps_m[:],
                                                scalar1=bqk_sb[:, m:m + 1])

                wv = []
                for k in range(NKT):
                    w = p1.tile([128, C], f32r, name=f"wv{k}")
                    nc.sync.dma_start(
                        out=w[:],
                        in_=w_attn[k * 128:(k + 1) * 128, 2 * C:3 * C].bitcast(f32r))
                    wv.append(w)
                v_send = p1.tile([128, NTB, C], f32r, name="v_send")
                for tb in range(NTB):
                    for nf in range(2):
                        nfw = 512 if nf == 0 else 256
                        nfo = nf * 512
                        ps_v = ps1.tile([128, 512], f32, name=f"v{tb}_{nf}",
                                        tag="vps")
                        for k in range(NKT):
                            nc.tensor.matmul(
                                ps_v[:, 0:nfw],
                                ln1T[:, k, tb * 128:(tb + 1) * 128],
                                wv[k][:, nfo:nfo + nfw],
                                start=(k == 0), stop=(k == NKT - 1))
                        nc.vector.tensor_add(out=v_send[:, tb, nfo:nfo + nfw],
                                             in0=ps_v[:, 0:nfw],
                                             in1=bv_bc[:, nfo:nfo + nfw])

                nc.sync.dma_start(
                    out=ag_in[0:KT_WORDS].rearrange("(m p t) -> p m t",
                                                    p=128, t=TL),
                    in_=kT_send[:])
                nc.sync.dma_start(
                    out=ag_in[KT_WORDS:BLOB].rearrange("(tb p f) -> p tb f",
                                                       p=128, f=C),
                    in_=v_send[:])

            nc.gpsimd.collective_compute(
                "AllGather", ALU.bypass,
                replica_groups=[list(range(N_CORES))],
                ins=[ag_in.opt()], outs=[ag_out.opt()])

            # ---------------- phase 6: attention ------------------------
            with tc.tile_pool(name=f"att{rep}", bufs=2) as att, \
                 tc.tile_pool(name=f"attn1{rep}", bufs=1) as att1, \
                 tc.tile_pool(name=f"psa{rep}", bufs=1, space="PSUM") as psa, \
                 tc.tile_pool(name=f"psb{rep}", bufs=2, space="PSUM") as psb:
                for hp in range(NPAIR):
                    av = [psa.tile([65, TL], f32, name=f"av{h}_{hp}",
                                   tag=f"av{h}") for h in range(2)]
                    for r in range(N_CORES):
                        ktb = att.tile([128, TL], f32r, name=f"ktb{hp}_{r}",
                                       tag="ktb")
                        nc.sync.dma_start(
                            out=ktb[:],
                            in_=ag_out[r, hp * 128 * TL:(hp + 1) * 128 * TL]
                            .rearrange("(p t) -> p t", t=TL))
                        vb = att.tile([128, NTB, 130], f32r,
                                      name=f"vb{hp}_{r}", tag="vb")
                        vb4 = vb.rearrange("p j (hh s) -> p j hh s", s=65)
                        nc.vector.memset(vb4[:, :, :, 64:65].bitcast(f32), 1.0)
                        v_all = ag_out[r, KT_WORDS:BLOB].rearrange(
                            "(j p hh d) -> p j hh d", j=NTB, p=128, d=D)
                        for hh in range(2):
                            nc.sync.dma_start(
                                out=vb4[:, :, hh, 0:64],
                                in_=v_all[:, :, 2 * hp + hh, :])
                        for half in range(2):
                            for h in range(2):
                                sc = psb.tile([128, 2, TL], f32,
                                              name=f"sc{h}_{hp}_{r}_{half}",
                                              tag=f"sc{h}", bufs=1)
                                for jj in range(2):
                                    j = half * 2 + jj
                                    nc.tensor.matmul(
                                        sc[:, jj, :],
                                        ktb[h * 64:(h + 1) * 64,
                                            j * 128:(j + 1) * 128],
                                        qT[h * 64:(h + 1) * 64, hp, :],
                                        start=True, stop=True,
                                        tile_position=(h * 64, 0))
                                ex = att.tile([128, 2, TL], f32r,
                                              name=f"ex{h}_{hp}_{r}_{half}",
                                              tag=f"ex{h}")
                                nc.scalar.activation(out=ex[:], in_=sc[:],
                                                     func=AF.Exp, scale=0.125)
                                for jj in range(2):
                                    j = half * 2 + jj
                                    nc.tensor.matmul(
                                        av[h][:],
                                        vb[:, j, h * 65:(h + 1) * 65],
                                        ex[:, jj, :],
                                        start=(r == 0 and j == 0),
                                        stop=(r == N_CORES - 1 and j == NTB - 1),
                                        skip_group_check=True)
                    # normalize pair, assemble yT tile hp
                    for h in range(2):
                        rd_f = att1.tile([1, TL], f32, name=f"rdf{h}_{hp}",
                                         tag=f"rdf{h}")
                        nc.vector.reciprocal(out=rd_f[:], in_=av[h][64:65, :])
                        rd_r = att1.tile([1, TL], f32r, name=f"rdr{h}_{hp}",
                                         tag=f"rdr{h}")
                        nc.vector.tensor_copy(out=rd_r[:], in_=rd_f[:])
                        bc = psb.tile([128, TL], f32, name=f"bc{h}_{hp}",
                                      tag="bc", bufs=2)
                        nc.tensor.matmul(bc[:], ones1[:], rd_r[:],
                                         start=True, stop=True)
                        av_sb = att1.tile([64, TL], f32, name=f"avsb{h}_{hp}",
                                          tag=f"avsb{h}")
                        nc.vector.tensor_copy(out=av_sb[:], in_=av[h][0:64, :])
                        if h == 0:
                            nc.vector.tensor_mul(out=yT[0:64, hp, :],
                                                 in0=av_sb[:],
                                                 in1=bc[0:64, :])
                        else:
                            y2 = att1.tile([64, TL], f32r, name=f"y2_{hp}",
                                           tag="y2")
                            nc.vector.tensor_mul(out=y2[:], in0=av_sb[:],
                                                 in1=bc[0:64, :])
                            nc.sync.dma_start(out=yT[64:128, hp, :], in_=y2[:])

            # ---------------- phase 7-8: out proj + residual ------------
            with tc.tile_pool(name=f"pr{rep}", bufs=1) as prp, \
                 tc.tile_pool(name=f"prs{rep}", bufs=2) as prs, \
                 tc.tile_pool(name=f"psp{rep}", bufs=2, space="PSUM") as psp:
                wao = []
                for k in range(NKT):
                    w = prp.tile([128, C], f32r, name=f"wao{k}")
                    nc.sync.dma_start(
                        out=w[:], in_=w_ao[k * 128:(k + 1) * 128, :].bitcast(f32r))
                    wao.append(w)
                for m in range(NKT):
                    ps_m = psp.tile([128, TL], f32, name=f"pr{m}", tag="prps")
                    for k in range(NKT):
                        nc.tensor.matmul(ps_m[:], wao[k][:, m * 128:(m + 1) * 128],
                                         yT[:, k, :],
                                         start=(k == 0), stop=(k == NKT - 1))
                    ap_sb = prs.tile([128, TL], f32, name=f"ap{m}", tag="apsb")
                    nc.vector.tensor_scalar_add(out=ap_sb[:], in0=ps_m[:],
                                                scalar1=bao_sb[:, m:m + 1])
                    for tb in range(NTB):
                        tp = psp.tile([128, 128], f32, name=f"tpr{m}_{tb}",
                                      tag="tpr")
                        nc.tensor.transpose(tp[:], ap_sb[:, tb * 128:(tb + 1) * 128],
                                            ident[:])
                        nc.vector.tensor_add(
                            out=x2_t[:, tb, m * 128:(m + 1) * 128],
                            in0=tp[:], in1=x_t[:, tb, m * 128:(m + 1) * 128])

            # ---------------- phase 9-11: LN2 + MLP ---------------------
            with tc.tile_pool(name=f"mlp{rep}", bufs=1) as mp, \
                 tc.tile_pool(name=f"mlps{rep}", bufs=2) as mps, \
                 tc.tile_pool(name=f"mlpw{rep}", bufs=6) as mpw, \
                 tc.tile_pool(name=f"psm{rep}", bufs=2, space="PSUM") as psm, \
                 tc.tile_pool(name=f"psq{rep}", bufs=1, space="PSUM") as psq:
                ln2 = mp.tile([128, NTB, C], f32, name="ln2")
                _layernorm_nat(nc, mps, x2_t, g2_bc, b2_bc, eps_t, ln2, "ln2")
                ln2T = mp.tile([128, NKT, TL], f32r, name="ln2T")
                _transpose_to_T(nc, psm, ln2, ln2T, ident, "t2")

                hT = mp.tile([128, NHT, TL], f32r, name="hT")
                for halfm in range(2):
                    wfc = []
                    for k in range(NKT):
                        w = mpw.tile([128, 12 * 128], f32r,
                                     name=f"wfc{halfm}_{k}", tag="wfc")
                        nc.sync.dma_start(
                            out=w[:],
                            in_=w_fc[k * 128:(k + 1) * 128,
                                     halfm * 1536:(halfm + 1) * 1536]
                            .bitcast(f32r))
                        wfc.append(w)
                    for mm in range(12):
                        m = halfm * 12 + mm
                        ps_m = psm.tile([128, TL], f32, name=f"fc{m}", tag="tp")
                        for k in range(NKT):
                            nc.tensor.matmul(
                                ps_m[:], wfc[k][:, mm * 128:(mm + 1) * 128],
                                ln2T[:, k, :],
                                start=(k == 0), stop=(k == NKT - 1))
                        nc.scalar.activation(out=hT[:, m, :], in_=ps_m[:],
                                             func=AF.Gelu,
                                             bias=bfc_sb[:, m:m + 1], scale=1.0)

                po = [psq.tile([128, TL], f32, name=f"po{m}", tag=f"po{m}")
                      for m in range(NKT)]
                for k in range(NHT):
                    wp = mps.tile([128, C], f32r, name=f"wp{k}", tag="wp", bufs=3)
                    nc.sync.dma_start(
                        out=wp[:],
                        in_=w_proj[k * 128:(k + 1) * 128, :].bitcast(f32r))
                    for m in range(NKT):
                        nc.tensor.matmul(po[m][:], wp[:, m * 128:(m + 1) * 128],
                                         hT[:, k, :],
                                         start=(k == 0), stop=(k == NHT - 1),
                                         skip_group_check=True)
                out_t = mp.tile([128, NTB, C], f32, name="out_t")
                for m in range(NKT):
                    ot_sb = mps.tile([128, TL], f32, name=f"ot{m}", tag="otsb")
                    nc.vector.tensor_scalar_add(out=ot_sb[:], in0=po[m][:],
                                                scalar1=bpr_sb[:, m:m + 1])
                    for tb in range(NTB):
                        tp = psm.tile([128, 128], f32, name=f"tpo{m}_{tb}",
                                      tag="tp")
                        nc.tensor.transpose(tp[:], ot_sb[:, tb * 128:(tb + 1) * 128],
                                            ident[:])
                        nc.vector.tensor_add(
                            out=out_t[:, tb, m * 128:(m + 1) * 128],
                            in0=tp[:], in1=x2_t[:, tb, m * 128:(m + 1) * 128])
                nc.sync.dma_start(
                    out=out.ap().rearrange("(tb p) f -> p tb f", p=128),
                    in_=out_t[:])

    nc.compile()
    return nc


_NC_CACHE = {}


def _get_nc(reps=1):
    if reps not in _NC_CACHE:
        _NC_CACHE[reps] = build(reps)
    return _NC_CACHE[reps]


def kernel(**inputs):
    x = np.asarray(inputs["x"])
    nc = _get_nc(1)
    shared = {k: np.ascontiguousarray(np.asarray(inputs[k]), dtype=np.float32)
              for k in ["w_attn", "b_attn", "w_ao", "b_ao", "ln1_g", "ln1_b",
                        "ln2_g", "ln2_b", "w_fc", "b_fc", "w_proj", "b_proj"]}
    in_maps = []
    for c in range(N_CORES):
        m = dict(shared)
        m["x_sh"] = np.ascontiguousarray(x[0, c * TL:(c + 1) * TL, :],
                                         dtype=np.float32)
        in_maps.append(m)
    res = run_bass_kernel_spmd(nc, in_maps, core_ids=list(range(N_CORES)))
    out = np.concatenate([res.results[c]["out_sh"] for c in range(N_CORES)],
                         axis=0)
    return out.reshape(1, T, C).astype(np.float32)
